# revision 1
# baseline (speedup 1.0000x reference)
"""Trainium2 Bass kernel for MultiHeadedSelfAttention with Shaw relative
position embeddings (clipped, R=64), sharded over 8 NeuronCores.

Sharding: core c handles batch b = c//4 and head group g = c%4 (4 heads).
Each core computes a partial output  ctx_g @ W_out[256g:256g+256]  for its
batch; the host sums the 4 partials per batch and adds b_out.
"""
import sys

sys.path.insert(0, "/opt/trn_rl_repo")

import numpy as np

B, S, D, H, RR, VOC = 2, 2048, 1024, 16, 64, 129
HD = 64              # head dim
NH = 4               # heads per core
N_CORES = 8
NT = S // 128        # 16 q-tiles of 128
NKT = S // 128       # 16 k-tiles
IMW = 512            # qrel image width (clip-padded)
IMWW = 384           # attn/cumsum image width (W-zone grid)
SCALE = 0.125        # 1/sqrt(64)

_cache = {}


def _regions(t):
    """W-zone bounds for q-tile t."""
    i0 = 128 * t
    wlo = max(0, i0 - 128)
    whi = min(S, i0 + 256)
    return i0, wlo, whi


def _build():
    import concourse.bass as bass
    import concourse.mybir as mybir
    import concourse.tile as tile
    from concourse import bacc
    from concourse.masks import make_identity
    from contextlib import ExitStack

    F32 = mybir.dt.float32
    F32R = mybir.dt.float32r
    F16 = mybir.dt.float16
    AP = bass.AP
    AF = mybir.ActivationFunctionType
    ALU = mybir.AluOpType

    nc = bacc.Bacc("TRN2", target_bir_lowering=False, debug=False,
                   num_devices=N_CORES)

    # ---------------- DRAM I/O ----------------
    xT = nc.dram_tensor("xT", [D, S], F32, kind="ExternalInput").ap()
    wq = nc.dram_tensor("wq", [D, 256], F32, kind="ExternalInput").ap()
    wk = nc.dram_tensor("wk", [D, 256], F32, kind="ExternalInput").ap()
    wv = nc.dram_tensor("wv", [D, 256], F32, kind="ExternalInput").ap()
    bq = nc.dram_tensor("bq", [128, 2], F32, kind="ExternalInput").ap()
    bk = nc.dram_tensor("bk", [128, 2], F32, kind="ExternalInput").ap()
    bv = nc.dram_tensor("bv", [128, 2], F32, kind="ExternalInput").ap()
    r01 = nc.dram_tensor("r01", [128, 2], F32, kind="ExternalInput").ap()
    relk = nc.dram_tensor("relk", [128, 132], F16, kind="ExternalInput").ap()
    rvm = nc.dram_tensor("rvm", [128, 64], F16, kind="ExternalInput").ap()
    rvl = nc.dram_tensor("rvl", [1, 64], F16, kind="ExternalInput").ap()
    wout = nc.dram_tensor("wout", [128, 2, 1024], F16, kind="ExternalInput").ap()
    out = nc.dram_tensor("out", [S, D], F32, kind="ExternalOutput").ap()

    # DRAM scratch images (per head, per q-tile blocks)
    imgq_t = nc.dram_tensor("imgq", [NH * NT * 128 * IMW], F16)
    imgw_t = nc.dram_tensor("imgw", [NH * NT * 128 * IMWW], F16)
    imgc_t = nc.dram_tensor("imgc", [NH * NT * 128 * IMWW], F16)

    with tile.TileContext(nc) as tc, ExitStack() as ctx:
        # ---------------- persistent pools ----------------
        pp = ctx.enter_context(tc.tile_pool(name="persist", bufs=1))
        qkT = []   # per pair: qT16, kW16, kL16, kR16  [128, S] fp16
        for pair in range(2):
            qkT.append({
                "q": pp.tile([128, S], F16, tag=f"qT{pair}", name=f"qT{pair}"),
                "W": pp.tile([128, S], F16, tag=f"kW{pair}", name=f"kW{pair}"),
                "L": pp.tile([128, S], F16, tag=f"kL{pair}", name=f"kL{pair}"),
                "R": pp.tile([128, S], F16, tag=f"kR{pair}", name=f"kR{pair}"),
            })
        v16 = pp.tile([128, NKT, 256], F16, tag="v16", name="v16")
        relk_sb = pp.tile([128, 132], F16, tag="relk", name="relk")
        rvm_sb = pp.tile([128, 64], F16, tag="rvm", name="rvm")
        rvl_sb = pp.tile([1, 64], F16, tag="rvl", name="rvl")
        wout_sb = pp.tile([128, 2, 1024], F16, tag="wout", name="wout")
        bq_sb = pp.tile([128, 2], F32, tag="bq", name="bq")
        bk_sb = pp.tile([128, 2], F32, tag="bk", name="bk")
        bv_sb = pp.tile([128, 2], F32, tag="bv", name="bv")
        r01_sb = pp.tile([128, 2], F32, tag="r01", name="r01")
        ones1 = pp.tile([1, 128], F16, tag="ones1", name="ones1")
        zeros16 = pp.tile([128, 128], F16, tag="zeros16", name="zeros16")
        # attnT pool: per head [128 k, kt, 512 q-group]  fp16
        attnT = [pp.tile([128, NKT, 512], F16, tag=f"attnT{h}", name=f"attnT{h}") for h in range(NH)]
        arelT = [pp.tile([128, 2, 512], F16, tag=f"arelT{h}", name=f"arelT{h}") for h in range(NH)]

        nc.sync.dma_start(relk_sb[:], relk)
        nc.sync.dma_start(rvm_sb[:], rvm)
        nc.sync.dma_start(rvl_sb[:], rvl)
        nc.sync.dma_start(wout_sb[:], wout)
        nc.sync.dma_start(bq_sb[:], bq)
        nc.sync.dma_start(bk_sb[:], bk)
        nc.sync.dma_start(bv_sb[:], bv)
        nc.sync.dma_start(r01_sb[:], r01)
        nc.gpsimd.memset(ones1[:], 1.0)
        nc.gpsimd.memset(zeros16[:], 0.0)

        # ---------------- phase 1: projections ----------------
        with tc.tile_pool(name="p1", bufs=1) as p1, \
             tc.tile_pool(name="p1ps", bufs=2, space="PSUM") as p1ps:
            xT_sb = p1.tile([128, 8, S], F32R, tag="xT", name="xT")
            wq_sb = p1.tile([128, 8, 256], F32R, tag="wq", name="wq")
            wk_sb = p1.tile([128, 8, 256], F32R, tag="wk", name="wk")
            wv_sb = p1.tile([128, 8, 256], F32R, tag="wv", name="wv")
            nc.sync.dma_start(xT_sb[:], xT.rearrange("(c p) s -> p c s", p=128).bitcast(F32R))
            nc.sync.dma_start(wq_sb[:], wq.rearrange("(c p) n -> p c n", p=128).bitcast(F32R))
            nc.sync.dma_start(wk_sb[:], wk.rearrange("(c p) n -> p c n", p=128).bitcast(F32R))
            nc.sync.dma_start(wv_sb[:], wv.rearrange("(c p) n -> p c n", p=128).bitcast(F32R))

            # q, k (transposed layout [col, s]) per pair
            for pair in range(2):
                for sc in range(4):  # s-chunks of 512
                    ps_q = p1ps.tile([128, 512], F32, tag="p1q", name="p1q")
                    ps_k = p1ps.tile([128, 512], F32, tag="p1k", name="p1k")
                    for dk in range(8):
                        nc.tensor.matmul(
                            ps_q[:], wq_sb[:, dk, 128 * pair:128 * pair + 128],
                            xT_sb[:, dk, 512 * sc:512 * sc + 512],
                            start=(dk == 0), stop=(dk == 7))
                        nc.tensor.matmul(
                            ps_k[:], wk_sb[:, dk, 128 * pair:128 * pair + 128],
                            xT_sb[:, dk, 512 * sc:512 * sc + 512],
                            start=(dk == 0), stop=(dk == 7))
                    cs = slice(512 * sc, 512 * sc + 512)
                    # qT16 = (psum + bq) * SCALE ; kW16 = psum + bk
                    nc.vector.tensor_scalar(
                        qkT[pair]["q"][:, cs], ps_q[:], bq_sb[:, pair:pair + 1],
                        SCALE, op0=ALU.add, op1=ALU.mult)
                    nc.vector.tensor_scalar_add(
                        qkT[pair]["W"][:, cs], ps_k[:], bk_sb[:, pair:pair + 1])
                    nc.vector.tensor_scalar_add(
                        qkT[pair]["L"][:, cs], qkT[pair]["W"][:, cs],
                        r01_sb[:, 0:1])
                    nc.vector.tensor_scalar_add(
                        qkT[pair]["R"][:, cs], qkT[pair]["W"][:, cs],
                        r01_sb[:, 1:2])

            # v (natural layout [s, col])
            for st in range(NT):
                ps_v = p1ps.tile([128, 256], F32, tag="p1v", name="p1v")
                for dk in range(8):
                    nc.tensor.matmul(
                        ps_v[:], xT_sb[:, dk, 128 * st:128 * st + 128],
                        wv_sb[:, dk, :], start=(dk == 0), stop=(dk == 7))
                nc.vector.tensor_copy(v16[:, st, :], ps_v[:])

        # ---------------- phase 2: attention ----------------
        p2 = ctx.enter_context(tc.tile_pool(name="p2", bufs=1))
        scps = ctx.enter_context(tc.tile_pool(name="scps", bufs=2, space="PSUM"))
        avps = ctx.enter_context(tc.tile_pool(name="avps", bufs=1, space="PSUM"))
        smps = ctx.enter_context(tc.tile_pool(name="smps", bufs=2, space="PSUM"))
        expp = ctx.enter_context(tc.tile_pool(name="expp", bufs=5))
        smal = ctx.enter_context(tc.tile_pool(name="smal", bufs=4))

        def img_base(handle, h, t, w):
            return (h * NT + t) * 128 * w

        for go in range(4):            # q512 groups
            # fp16 reciprocals laid out [1, head*512 + q] for K=1 broadcast
            recipT16 = smal.tile([1, 4 * 512], F16, tag="recipT16", name="recipT16")
            for tq in range(4):        # q128 tiles within group
                t = 4 * go + tq
                i0, wlo, whi = _regions(t)
                wlen = whi - wlo
                moff = wlo - (i0 - 128)      # attnW grid offset (0 or 128)
                recips = smal.tile([128, 4], F32, tag="recips", name="recips")
                for pair in range(2):
                    # ---- Qrel (both heads, row-tiled) -> qrel image
                    qrps = [smps.tile([128, 132], F32, tag="sm", name="sm") for _ in range(2)]
                    for h01 in range(2):
                        rs = slice(64 * h01, 64 * h01 + 64)
                        nc.tensor.matmul(
                            qrps[h01][:], qkT[pair]["q"][rs, 128 * t:128 * t + 128],
                            relk_sb[rs, :], start=True, stop=True)
                    for h01 in range(2):
                        h = 2 * pair + h01
                        qrelpad = smal.tile([128, IMW], F16, tag="qrelpad", name="qrelpad")
                        nc.vector.tensor_copy(qrelpad[:, 192:321], qrps[h01][:, 0:129])
                        nc.vector.tensor_copy(
                            qrelpad[:, 0:192],
                            qrps[h01][:, 0:1].broadcast_to([128, 192]))
                        nc.vector.tensor_copy(
                            qrelpad[:, 321:IMW],
                            qrps[h01][:, 128:129].broadcast_to([128, IMW - 321]))
                        b = img_base(imgq_t, h, t, IMW)
                        nc.sync.dma_start(
                            AP(imgq_t, b, [[IMW, 128], [1, IMW]]), qrelpad[:])

                    # ---- scores + band + exp, per 512-col chunk
                    bands = {}
                    exps = {}
                    accs = {}
                    for h01 in range(2):
                        h = 2 * pair + h01
                        bq_ = img_base(imgq_t, h, t, IMW)
                        band = smal.tile([128, wlen], F16, tag="band", name="band")
                        nc.sync.dma_start(
                            band[:],
                            AP(imgq_t, bq_ + 256 + wlo - i0, [[IMW - 1, 128], [1, wlen]]))
                        bands[h01] = band
                        exps[h01] = expp.tile([128, S], F16, tag="exp", name="exp")
                        accs[h01] = smal.tile([128, 4], F32, tag="acc", name="acc")

                    for c in range(4):   # 512-col chunks
                        clo, chi = 512 * c, 512 * c + 512
                        sc_ps = [scps.tile([128, 512], F32, tag=f"sc{h01}", name=f"sc{h01}")
                                 for h01 in range(2)]
                        # region pieces within this chunk
                        pieces = []
                        for rlo, rhi, key in ((0, wlo, "L"), (wlo, whi, "W"),
                                              (whi, S, "R")):
                            lo, hi = max(rlo, clo), min(rhi, chi)
                            if lo < hi:
                                pieces.append((lo, hi, key))
                        for h01 in range(2):
                            rs = slice(64 * h01, 64 * h01 + 64)
                            for lo, hi, key in pieces:
                                nc.tensor.matmul(
                                    sc_ps[h01][:, lo - clo:hi - clo],
                                    qkT[pair]["q"][rs, 128 * t:128 * t + 128],
                                    qkT[pair][key][rs, lo:hi],
                                    start=True, stop=True)
                        # band add (W-zone part of chunk)
                        blo, bhi = max(wlo, clo), min(whi, chi)
                        for h01 in range(2):
                            if blo < bhi:
                                nc.vector.tensor_add(
                                    sc_ps[h01][:, blo - clo:bhi - clo],
                                    sc_ps[h01][:, blo - clo:bhi - clo],
                                    bands[h01][:, blo - wlo:bhi - wlo])
                            nc.scalar.activation(
                                exps[h01][:, clo:chi], sc_ps[h01][:], AF.Exp,
                                accum_out=accs[h01][:, c:c + 1])

                    # ---- per-head epilogue: cumsum, images, Arel, recip
                    for h01 in range(2):
                        h = 2 * pair + h01
                        ex = exps[h01]
                        acc = accs[h01]
                        Cpad = smal.tile([128, IMWW], F16, tag="Cpad", name="Cpad")
                        C16 = Cpad[:, moff:moff + wlen]
                        nc.vector.tensor_tensor_scan(
                            C16, ex[:, wlo:whi], ex[:, wlo:whi], 0.0,
                            op0=ALU.add, op1=ALU.bypass)
                        bw = img_base(imgw_t, h, t, IMWW)
                        bc = img_base(imgc_t, h, t, IMWW)
                        nc.sync.dma_start(
                            AP(imgw_t, bw + moff, [[IMWW, 128], [1, wlen]]),
                            ex[:, wlo:whi])
                        if moff > 0:   # t == 0: zero guards on the left
                            nc.sync.dma_start(
                                AP(imgw_t, bw, [[IMWW, 128], [1, moff]]),
                                zeros16[:, 0:moff])
                            nc.vector.tensor_copy(Cpad[:, 0:moff],
                                                  zeros16[:, 0:moff])
                        if moff + wlen < IMWW:  # t == 15: right guards
                            gl = IMWW - (moff + wlen)
                            nc.sync.dma_start(
                                AP(imgw_t, bw + moff + wlen, [[IMWW, 128], [1, gl]]),
                                zeros16[:, 0:gl])
                            # cumsum saturates: right guard = row total
                            nc.vector.tensor_copy(
                                Cpad[:, moff + wlen:IMWW],
                                C16[:, wlen - 1:wlen].broadcast_to([128, gl]))
                        nc.sync.dma_start(
                            AP(imgc_t, bc, [[IMWW, 128], [1, IMWW]]), Cpad[:])

                        # D = sum of chunk accums
                        Dt = smal.tile([128, 1], F32, tag="Dt", name="Dt")
                        nc.vector.tensor_reduce(
                            Dt[:], acc[:], axis=mybir.AxisListType.X, op=ALU.add)

                        # S_L from chunk accums minus in-chunk W mass
                        sL = smal.tile([128, 1], F32, tag="sL", name="sL")
                        if wlo == 0:
                            nc.vector.memset(sL[:], 0.0)
                        else:
                            nfull = wlo // 512
                            r = wlo % 512
                            terms = [acc[:, c:c + 1] for c in range(nfull)]
                            if nfull == 1:
                                nc.vector.tensor_copy(sL[:], terms[0])
                            elif nfull > 1:
                                nc.vector.tensor_reduce(
                                    sL[:], acc[:, 0:nfull],
                                    axis=mybir.AxisListType.X, op=ALU.add)
                            if r > 0:
                                # straddle chunk nfull: L part = a - Wmass
                                mhi = min(512 - r, wlen)   # W cols [0, mhi) in chunk
                                tmp = smal.tile([128, 1], F32, tag="tmp1", name="tmp1")
                                nc.vector.tensor_tensor(
                                    tmp[:], acc[:, nfull:nfull + 1],
                                    C16[:, mhi - 1:mhi], op=ALU.subtract)
                                if nfull == 0:
                                    nc.vector.tensor_copy(sL[:], tmp[:])
                                else:
                                    nc.vector.tensor_add(sL[:], sL[:], tmp[:])

                        # skew reads: arel central + c0/c1
                        arel16 = smal.tile([128, 256], F16, tag="arel16", name="arel16")
                        nc.vector.memset(arel16[:, 129:256], 0.0)
                        nc.sync.dma_start(
                            arel16[:, 1:128],
                            AP(imgw_t, bw + 65, [[IMWW + 1, 128], [1, 127]]))
                        c0 = smal.tile([128, 2], F16, tag="c01", name="c01")
                        nc.sync.dma_start(
                            c0[:, 0:1],
                            AP(imgc_t, bc + 64, [[IMWW + 1, 128], [1, 1]]))
                        nc.sync.dma_start(
                            c0[:, 1:2],
                            AP(imgc_t, bc + 191, [[IMWW + 1, 128], [1, 1]]))
                        nc.vector.tensor_tensor(
                            arel16[:, 0:1], sL[:], c0[:, 0:1], op=ALU.add)
                        tmp2 = smal.tile([128, 1], F32, tag="tmp2", name="tmp2")
                        nc.vector.tensor_tensor(
                            tmp2[:], Dt[:], sL[:], op=ALU.subtract)
                        nc.vector.tensor_tensor(
                            arel16[:, 128:129], tmp2[:], c0[:, 1:2],
                            op=ALU.subtract)

                        # transposes
                        nc.sync.dma_start_transpose(
                            attnT[h][:, :, 128 * tq:128 * tq + 128], ex[:])
                        nc.sync.dma_start_transpose(
                            arelT[h][:, :, 128 * tq:128 * tq + 128], arel16[:])

                        # reciprocal of D into recips col h
                        nc.vector.reciprocal(recips[:, h:h + 1], Dt[:])

                # scatter recips [128 q, 4 h] -> recipT16 [1, 512h + 128tq + q]
                recips16 = smal.tile([128, 4], F16, tag="recips16", name="recips16")
                nc.vector.tensor_copy(recips16[:], recips[:])
                for h in range(4):
                    o = 512 * h + 128 * tq
                    nc.sync.dma_start(recipT16[0:1, o:o + 128],
                                      recips16[:, h:h + 1])

            # ---------------- per-group AV + rel-v + normalize + out ----
            ctx16 = {}
            for pair in range(2):
                # broadcast recip rows via K=1 outer product (fp16)
                bc_ps = smps.tile([128, 512], F32, tag="sm", name="sm")
                for h01 in range(2):
                    h = 2 * pair + h01
                    nc.tensor.matmul(
                        bc_ps[64 * h01:64 * h01 + 64, :], ones1[0:1, 0:64],
                        recipT16[0:1, 512 * h:512 * h + 512], start=True,
                        stop=True, tile_position=(0, 64 * h01))
                rbc = smal.tile([128, 512], F32, tag="rbc", name="rbc")
                nc.vector.tensor_copy(rbc[:], bc_ps[:])

                ctx_ps = avps.tile([128, 512], F32, tag=f"av{pair}", name=f"av{pair}")
                for h01 in range(2):
                    h = 2 * pair + h01
                    cs = slice(64 * h01, 64 * h01 + 64)
                    tp = (0, 64 * h01)
                    for kt in range(NKT):
                        nc.tensor.matmul(
                            ctx_ps[cs, :], v16[:, kt, 64 * h:64 * h + 64],
                            attnT[h][:, kt, :], start=(kt == 0), stop=False,
                            tile_position=tp)
                    nc.tensor.matmul(
                        ctx_ps[cs, :], rvm_sb[:, :], arelT[h][:, 0, :],
                        start=False, stop=False, tile_position=tp)
                    nc.tensor.matmul(
                        ctx_ps[cs, :], rvl_sb[0:1, :], arelT[h][0:1, 1, :],
                        start=False, stop=True, tile_position=tp)

                ct = p2.tile([128, 512], F16, tag=f"ctx16_{pair}", name=f"ctx16_{pair}")
                nc.vector.tensor_tensor(ct[:], ctx_ps[:], rbc[:], op=ALU.mult)
                nc.vector.tensor_scalar_add(ct[:], ct[:], bv_sb[:, pair:pair + 1])
                ctx16[pair] = ct

            # ---- output projection for this q512 group
            for tq in range(4):
                out_sb = smal.tile([128, 1024], F32, tag="out_sb", name="out_sb")
                for nch in range(2):
                    op_ps = smps.tile([128, 512], F32, tag="sm", name="sm")
                    for pair in range(2):
                        nc.tensor.matmul(
                            op_ps[:], ctx16[pair][:, 128 * tq:128 * tq + 128],
                            wout_sb[:, pair, 512 * nch:512 * nch + 512],
                            start=(pair == 0), stop=(pair == 1))
                    nc.vector.tensor_copy(
                        out_sb[:, 512 * nch:512 * nch + 512], op_ps[:])
                r0_ = 512 * go + 128 * tq
                nc.sync.dma_start(out[r0_:r0_ + 128, :], out_sb[:])

    nc.compile()
    return nc


def get_nc():
    if "nc" not in _cache:
        _cache["nc"] = _build()
    return _cache["nc"]


def shard_inputs(inputs):
    """Build per-core input maps from full inputs (layout prep only)."""
    x = np.asarray(inputs["x"], np.float32)
    W_qkv = np.asarray(inputs["W_qkv"], np.float32)
    b_qkv = np.asarray(inputs["b_qkv"], np.float32)
    W_out = np.asarray(inputs["W_out"], np.float32)
    rk = np.asarray(inputs["rel_emb_k"], np.float32)
    rv = np.asarray(inputs["rel_emb_v"], np.float32)

    Wq, Wk, Wv = W_qkv[:, 0:D], W_qkv[:, D:2 * D], W_qkv[:, 2 * D:3 * D]
    bqf, bkf, bvf = b_qkv[0:D], b_qkv[D:2 * D], b_qkv[2 * D:3 * D]

    relk_host = np.zeros((128, 132), np.float16)
    relk_host[0:64, 0:129] = rk.T.astype(np.float16)
    relk_host[64:128, 0:129] = rk.T.astype(np.float16)
    rvm_host = rv[0:128].astype(np.float16)
    rvl_host = rv[128:129].astype(np.float16)
    r0 = np.tile(rk[0], 2).reshape(128, 1)
    r1 = np.tile(rk[128], 2).reshape(128, 1)
    r01_host = np.concatenate([r0, r1], 1).astype(np.float32)

    in_maps = []
    for c in range(N_CORES):
        b, g = c // 4, c % 4
        cols = slice(256 * g, 256 * g + 256)
        m = {
            "xT": np.ascontiguousarray(x[b].T),
            "wq": np.ascontiguousarray(Wq[:, cols]),
            "wk": np.ascontiguousarray(Wk[:, cols]),
            "wv": np.ascontiguousarray(Wv[:, cols]),
            "bq": np.ascontiguousarray(bqf[cols].reshape(2, 128).T),
            "bk": np.ascontiguousarray(bkf[cols].reshape(2, 128).T),
            "bv": np.ascontiguousarray(bvf[cols].reshape(2, 128).T),
            "r01": r01_host,
            "relk": relk_host,
            "rvm": rvm_host,
            "rvl": rvl_host,
            "wout": np.ascontiguousarray(
                W_out[cols].reshape(2, 128, 1024).transpose(1, 0, 2)
            ).astype(np.float16),
        }
        in_maps.append(m)
    return in_maps


def unshard_outputs(results, inputs):
    b_out = np.asarray(inputs["b_out"], np.float32)
    out = np.zeros((B, S, D), np.float32)
    for c in range(N_CORES):
        out[c // 4] += results[c]["out"]
    out += b_out[None, None, :]
    return out


def kernel(**inputs):
    from concourse import bass_utils
    nc = get_nc()
    in_maps = shard_inputs(inputs)
    res = bass_utils.run_bass_kernel_spmd(nc, in_maps, list(range(N_CORES)))
    return unshard_outputs(res.results, inputs)


if __name__ == "__main__":
    import json
    rng = np.random.default_rng(0)
    demo = {
        "x": rng.standard_normal((B, S, D)).astype(np.float32),
        "W_qkv": (rng.standard_normal((D, 3 * D)) * 0.02).astype(np.float32),
        "b_qkv": np.zeros(3 * D, np.float32),
        "W_out": (rng.standard_normal((D, D)) * 0.02).astype(np.float32),
        "b_out": np.zeros(D, np.float32),
        "rel_emb_k": (rng.standard_normal((VOC, HD)) * 0.02).astype(np.float32),
        "rel_emb_v": (rng.standard_normal((VOC, HD)) * 0.02).astype(np.float32),
    }
    o = kernel(**demo)
    print(o.shape, float(np.abs(o).max()))



# revision 15
# speedup vs baseline: 1.1908x; 1.1908x over previous
"""Trainium2 Bass kernel for MultiHeadedSelfAttention with Shaw relative
position embeddings (clipped, R=64), sharded over 8 NeuronCores.

Sharding: core c handles batch b = c//4 and head group g = c%4 (4 heads).
Each core computes a partial output  ctx_g @ W_out[256g:256g+256]  for its
batch; the host sums the 4 partials per batch and adds b_out.

v2: PE-array transposes of the attention matrix (replacing DMA
transposes), mask-based softmax tail sums (replacing the cumsum image),
merged/multi-engine DMA dispatch.
"""
import sys

sys.path.insert(0, "/opt/trn_rl_repo")

import numpy as np

B, S, D, H, RR, VOC = 2, 2048, 1024, 16, 64, 129
HD = 64              # head dim
NH = 4               # heads per core
N_CORES = 8
NT = S // 128        # 16 q-tiles of 128
NKT = S // 128       # 16 k-tiles
IMW = 512            # qrel image width (clip-padded)
IMWW = 448           # exp W-zone image width (64-col front pad)
FPAD = 64            # front pad of the exp image
SCALE = 0.125        # 1/sqrt(64)

_cache = {}


def _regions(t):
    """W-zone bounds for q-tile t."""
    i0 = 128 * t
    wlo = max(0, i0 - 128)
    whi = 256 if t == 0 else min(S, i0 + 256)
    return i0, wlo, whi


def _build():
    import concourse.bass as bass
    import concourse.mybir as mybir
    import concourse.tile as tile
    from concourse import bacc
    from contextlib import ExitStack

    F32 = mybir.dt.float32
    F32R = mybir.dt.float32r
    F16 = mybir.dt.float16
    AP = bass.AP
    AF = mybir.ActivationFunctionType
    ALU = mybir.AluOpType
    AX = mybir.AxisListType

    nc = bacc.Bacc("TRN2", target_bir_lowering=False, debug=False,
                   num_devices=N_CORES)

    # ---------------- DRAM I/O ----------------
    xT = nc.dram_tensor("xT", [D, S], F32, kind="ExternalInput").ap()
    wq = nc.dram_tensor("wq", [D, 256], F32, kind="ExternalInput").ap()
    wk = nc.dram_tensor("wk", [D, 256], F32, kind="ExternalInput").ap()
    wv = nc.dram_tensor("wv", [D, 256], F32, kind="ExternalInput").ap()
    bq = nc.dram_tensor("bq", [128, 2], F32, kind="ExternalInput").ap()
    bk = nc.dram_tensor("bk", [128, 2], F32, kind="ExternalInput").ap()
    bv = nc.dram_tensor("bv", [128, 2], F32, kind="ExternalInput").ap()
    r01 = nc.dram_tensor("r01", [128, 2], F32, kind="ExternalInput").ap()
    relk = nc.dram_tensor("relk", [128, 132], F16, kind="ExternalInput").ap()
    rvm = nc.dram_tensor("rvm", [128, 64], F16, kind="ExternalInput").ap()
    rvl = nc.dram_tensor("rvl", [1, 64], F16, kind="ExternalInput").ap()
    wout = nc.dram_tensor("wout", [128, 2, 1024], F16, kind="ExternalInput").ap()
    # tail masks: [ML | MR] x [t==0 | middle] fp16 [128, 384]
    mlm = nc.dram_tensor("mlm", [128, 512], F16, kind="ExternalInput").ap()
    mrm = nc.dram_tensor("mrm", [128, 512], F16, kind="ExternalInput").ap()
    ident = nc.dram_tensor("ident", [128, 128], F16, kind="ExternalInput").ap()
    out = nc.dram_tensor("out", [S, D], F32, kind="ExternalOutput").ap()

    # DRAM scratch images
    imgq_t = nc.dram_tensor("imgq", [NT * NH * 128 * IMW], F16)   # qrel pad
    imgw_t = nc.dram_tensor("imgw", [NT * NH * 128 * IMWW], F16)  # exp W-zone

    def qbase(t):
        return t * NH * 128 * IMW

    def wbase(t, h=0):
        return (t * NH + h) * 128 * IMWW

    with tile.TileContext(nc) as tc, ExitStack() as ctx:
        # ---------------- persistent pools ----------------
        pp = ctx.enter_context(tc.tile_pool(name="persist", bufs=1))
        qkT = []   # per pair: qT16, kW16, kL16, kR16  [128, S] fp16
        for pair in range(2):
            qkT.append({
                "q": pp.tile([128, S], F16, tag=f"qT{pair}", name=f"qT{pair}"),
                "W": pp.tile([128, S], F16, tag=f"kW{pair}", name=f"kW{pair}"),
                "L": pp.tile([128, S], F16, tag=f"kL{pair}", name=f"kL{pair}"),
                "R": pp.tile([128, S], F16, tag=f"kR{pair}", name=f"kR{pair}"),
            })
        v16 = pp.tile([128, NKT, 256], F16, tag="v16", name="v16")
        relk_sb = pp.tile([128, 132], F16, tag="relk", name="relk")
        rvm_sb = pp.tile([128, 64], F16, tag="rvm", name="rvm")
        rvl_sb = pp.tile([1, 64], F16, tag="rvl", name="rvl")
        wout_sb = pp.tile([128, 2, 1024], F16, tag="wout", name="wout")
        bq_sb = pp.tile([128, 2], F32, tag="bq", name="bq")
        bk_sb = pp.tile([128, 2], F32, tag="bk", name="bk")
        bv_sb = pp.tile([128, 2], F32, tag="bv", name="bv")
        r01_sb = pp.tile([128, 2], F32, tag="r01", name="r01")
        ml_sb = pp.tile([128, 512], F16, tag="ml", name="ml")
        mr_sb = pp.tile([128, 512], F16, tag="mr", name="mr")
        id_sb = pp.tile([128, 128], F16, tag="ident", name="ident")
        ones1 = pp.tile([1, 128], F16, tag="ones1", name="ones1")
        zeros16 = pp.tile([128, 128], F16, tag="zeros16", name="zeros16")
        attnT = [pp.tile([128, NKT, 512], F16, tag=f"attnT{h}", name=f"attnT{h}")
                 for h in range(NH)]
        arelT = [pp.tile([128, 512], F16, tag=f"arelT{h}", name=f"arelT{h}")
                 for h in range(NH)]

        nc.sync.dma_start(relk_sb[:], relk)
        nc.sync.dma_start(rvm_sb[:], rvm)
        nc.sync.dma_start(rvl_sb[:], rvl)
        nc.sync.dma_start(wout_sb[:], wout)
        nc.sync.dma_start(bq_sb[:], bq)
        nc.sync.dma_start(bk_sb[:], bk)
        nc.sync.dma_start(bv_sb[:], bv)
        nc.sync.dma_start(r01_sb[:], r01)
        nc.sync.dma_start(ml_sb[:], mlm)
        nc.sync.dma_start(mr_sb[:], mrm)
        nc.sync.dma_start(id_sb[:], ident)
        nc.gpsimd.memset(ones1[:], 1.0)
        nc.gpsimd.memset(zeros16[:], 0.0)
        # zero the t=0 front pad (cols [0,64)) and t=15 back pad
        # (cols [320,448)) of the exp images so diagonal reads see 0
        for h in range(NH):
            nc.sync.dma_start(
                AP(imgw_t, wbase(0, h), [[IMWW, 128], [1, FPAD]]),
                zeros16[:, 0:FPAD])
            nc.sync.dma_start(
                AP(imgw_t, wbase(15, h) + FPAD + 256,
                   [[IMWW, 128], [1, 128]]),
                zeros16[:, 0:128])

        # ---------------- phase 1: projections ----------------
        with tc.tile_pool(name="p1", bufs=1) as p1, \
             tc.tile_pool(name="p1ps", bufs=2, space="PSUM") as p1ps:
            xT_sb = p1.tile([128, 8, S], F32R, tag="xT", name="xT")
            wq_sb = p1.tile([128, 8, 256], F32R, tag="wq", name="wq")
            wk_sb = p1.tile([128, 8, 256], F32R, tag="wk", name="wk")
            wv_sb = p1.tile([128, 8, 256], F32R, tag="wv", name="wv")
            nc.sync.dma_start(xT_sb[:], xT.rearrange("(c p) s -> p c s", p=128).bitcast(F32R))
            nc.sync.dma_start(wq_sb[:], wq.rearrange("(c p) n -> p c n", p=128).bitcast(F32R))
            nc.sync.dma_start(wk_sb[:], wk.rearrange("(c p) n -> p c n", p=128).bitcast(F32R))
            nc.sync.dma_start(wv_sb[:], wv.rearrange("(c p) n -> p c n", p=128).bitcast(F32R))

            # q, k (transposed layout [col, s]) per pair
            for pair in range(2):
                for sc in range(4):  # s-chunks of 512
                    ps_q = p1ps.tile([128, 512], F32, tag="p1q", name="p1q")
                    ps_k = p1ps.tile([128, 512], F32, tag="p1k", name="p1k")
                    for dk in range(8):
                        nc.tensor.matmul(
                            ps_q[:], wq_sb[:, dk, 128 * pair:128 * pair + 128],
                            xT_sb[:, dk, 512 * sc:512 * sc + 512],
                            start=(dk == 0), stop=(dk == 7))
                        nc.tensor.matmul(
                            ps_k[:], wk_sb[:, dk, 128 * pair:128 * pair + 128],
                            xT_sb[:, dk, 512 * sc:512 * sc + 512],
                            start=(dk == 0), stop=(dk == 7))
                    cs = slice(512 * sc, 512 * sc + 512)
                    nc.vector.tensor_scalar(
                        qkT[pair]["q"][:, cs], ps_q[:], bq_sb[:, pair:pair + 1],
                        SCALE, op0=ALU.add, op1=ALU.mult)
                    nc.vector.tensor_scalar_add(
                        qkT[pair]["W"][:, cs], ps_k[:], bk_sb[:, pair:pair + 1])
                    nc.gpsimd.tensor_scalar_add(
                        qkT[pair]["L"][:, cs], qkT[pair]["W"][:, cs],
                        r01_sb[:, 0:1])
                    nc.gpsimd.tensor_scalar_add(
                        qkT[pair]["R"][:, cs], qkT[pair]["W"][:, cs],
                        r01_sb[:, 1:2])

            # v (natural layout [s, col])
            for st in range(NT):
                ps_v = p1ps.tile([128, 256], F32, tag="p1v", name="p1v")
                for dk in range(8):
                    nc.tensor.matmul(
                        ps_v[:], xT_sb[:, dk, 128 * st:128 * st + 128],
                        wv_sb[:, dk, :], start=(dk == 0), stop=(dk == 7))
                nc.vector.tensor_copy(v16[:, st, :], ps_v[:])

        # ---------------- phase 1.5: qrel images for all tiles ----------
        qrp = ctx.enter_context(tc.tile_pool(name="qrp", bufs=2))
        qrps = ctx.enter_context(tc.tile_pool(name="qrps", bufs=1, space="PSUM"))
        for t in range(NT):
            qrelpad = qrp.tile([128, NH, IMW], F16, tag="qrelpad", name="qrelpad")
            for pair in range(2):
                for h01 in range(2):
                    h = 2 * pair + h01
                    rs = slice(64 * h01, 64 * h01 + 64)
                    qr = qrps.tile([128, 132], F32, tag="qr", name="qr")
                    nc.tensor.matmul(
                        qr[:], qkT[pair]["q"][rs, 128 * t:128 * t + 128],
                        relk_sb[rs, :], start=True, stop=True)
                    qrs = qrp.tile([128, 132], F16, tag="qrs", name="qrs")
                    nc.vector.tensor_copy(qrs[:, 0:132], qr[:])
                    nc.gpsimd.tensor_copy(qrelpad[:, h, 192:321], qrs[:, 0:129])
                    nc.gpsimd.tensor_copy(
                        qrelpad[:, h, 0:192],
                        qrs[:, 0:1].broadcast_to([128, 192]))
                    nc.gpsimd.tensor_copy(
                        qrelpad[:, h, 321:IMW],
                        qrs[:, 128:129].broadcast_to([128, IMW - 321]))
            nc.sync.dma_start(
                AP(imgq_t, qbase(t), [[IMW, 128], [128 * IMW, NH], [1, IMW]]),
                qrelpad[:])

        # ---------------- phase 2: attention ----------------
        p2 = ctx.enter_context(tc.tile_pool(name="p2", bufs=1))
        scps = ctx.enter_context(tc.tile_pool(name="scps", bufs=2, space="PSUM"))
        tpps = ctx.enter_context(tc.tile_pool(name="tpps", bufs=2, space="PSUM"))
        avps = ctx.enter_context(tc.tile_pool(name="avps", bufs=1, space="PSUM"))
        smps = ctx.enter_context(tc.tile_pool(name="smps", bufs=1, space="PSUM"))
        expp = ctx.enter_context(tc.tile_pool(name="expp", bufs=2))
        smal = ctx.enter_context(tc.tile_pool(name="smal", bufs=4))
        big2 = ctx.enter_context(tc.tile_pool(name="big2", bufs=2))
        bndp = ctx.enter_context(tc.tile_pool(name="bndp", bufs=2))

        for go in range(4):            # q512 groups
            recipT16 = big2.tile([1, NH * 512], F16, tag="recipT16",
                                 name="recipT16")
            arelR = big2.tile([1, NH * 512], F16, tag="arelR", name="arelR")
            for tq in range(4):        # q128 tiles within group
                t = 4 * go + tq
                i0, wlo, whi = _regions(t)
                wlen = whi - wlo
                moff = 128 if t == 0 else 0   # t=0 masks = middle masks shifted

                # ---- band read (from qrel image): all 4 heads in one DMA
                band4 = bndp.tile([128, NH, 384], F16, tag="band4", name="band4")
                nc.sync.dma_start(
                    band4[:, :, 0:wlen],
                    AP(imgq_t, qbase(t) + 256 - (i0 - wlo),
                       [[IMW - 1, 128], [128 * IMW, NH], [1, wlen]]))

                recips = smal.tile([128, 4], F32, tag="recips", name="recips")
                pack8 = smal.tile([128, 8], F16, tag="pack8", name="pack8")
                arel16 = smal.tile([128, NH, 129], F16, tag="arel16",
                                   name="arel16")
                exps = {}
                for pair in range(2):
                    for h01 in range(2):
                        h = 2 * pair + h01
                        rs = slice(64 * h01, 64 * h01 + 64)
                        ex = expp.tile([128, S], F16, tag=f"exp{h}",
                                       name=f"exp{h}")
                        exps[h] = ex
                        acc = smal.tile([128, 4], F32, tag=f"acc{h}",
                                        name=f"acc{h}")

                        # ---- scores + band + exp, per 512-col chunk
                        for c in range(4):
                            clo, chi = 512 * c, 512 * c + 512
                            sc_ps = scps.tile([128, 512], F32, tag="sc",
                                              name="sc")
                            pieces = []
                            for rlo, rhi, key in ((0, wlo, "L"), (wlo, whi, "W"),
                                                  (whi, S, "R")):
                                lo, hi = max(rlo, clo), min(rhi, chi)
                                if lo < hi:
                                    pieces.append((lo, hi, key))
                            for lo, hi, key in pieces:
                                nc.tensor.matmul(
                                    sc_ps[:, lo - clo:hi - clo],
                                    qkT[pair]["q"][rs, 128 * t:128 * t + 128],
                                    qkT[pair][key][rs, lo:hi],
                                    start=True, stop=True)
                            blo, bhi = max(wlo, clo), min(whi, chi)
                            if blo < bhi:
                                nc.vector.tensor_add(
                                    sc_ps[:, blo - clo:bhi - clo],
                                    sc_ps[:, blo - clo:bhi - clo],
                                    band4[:, h, blo - wlo:bhi - wlo])
                            nc.scalar.activation(
                                ex[:, clo:chi], sc_ps[:], AF.Exp,
                                accum_out=acc[:, c:c + 1])

                        # ---- write exp W-zone to image (for arel diag read)
                        nc.sync.dma_start(
                            AP(imgw_t, wbase(t, h) + FPAD,
                               [[IMWW, 128], [1, wlen]]),
                            ex[:, wlo:whi])

                        # ---- masked tail sums
                        tails = smal.tile([128, 2], F32, tag=f"tails{h}",
                                          name=f"tails{h}")
                        scr = smal.tile([128, 384], F16, tag="scr", name="scr")
                        nc.vector.tensor_tensor(
                            scr[:, 0:wlen], ex[:, wlo:whi],
                            ml_sb[:, moff:moff + wlen], op=ALU.mult)
                        nc.vector.tensor_reduce(
                            tails[:, 0:1], scr[:, 0:wlen], axis=AX.X,
                            op=ALU.add)
                        scr2 = smal.tile([128, 384], F16, tag="scr", name="scr")
                        nc.vector.tensor_tensor(
                            scr2[:, 0:wlen], ex[:, wlo:whi],
                            mr_sb[:, moff:moff + wlen], op=ALU.mult)
                        nc.vector.tensor_reduce(
                            tails[:, 1:2], scr2[:, 0:wlen], axis=AX.X,
                            op=ALU.add)

                        # sL = full chunks + partial chunk left of wlo
                        sL = smal.tile([128, 2], F32, tag=f"sLR{h}",
                                       name=f"sLR{h}")
                        nfull = wlo // 512
                        rlen = wlo - 512 * nfull
                        if nfull == 0 and rlen == 0:
                            nc.vector.memset(sL[:, 0:1], 0.0)
                        elif rlen == 0:
                            if nfull == 1:
                                nc.vector.tensor_copy(sL[:, 0:1], acc[:, 0:1])
                            else:
                                nc.vector.tensor_reduce(
                                    sL[:, 0:1], acc[:, 0:nfull], axis=AX.X,
                                    op=ALU.add)
                        else:
                            nc.vector.tensor_reduce(
                                sL[:, 0:1], ex[:, 512 * nfull:wlo], axis=AX.X,
                                op=ALU.add)
                            if nfull == 1:
                                nc.vector.tensor_add(
                                    sL[:, 0:1], sL[:, 0:1], acc[:, 0:1])
                            elif nfull > 1:
                                tmp = smal.tile([128, 1], F32, tag="tmp1",
                                                name="tmp1")
                                nc.vector.tensor_reduce(
                                    tmp[:], acc[:, 0:nfull], axis=AX.X,
                                    op=ALU.add)
                                nc.vector.tensor_add(sL[:, 0:1], sL[:, 0:1],
                                                     tmp[:])
                        # sR = full chunks + partial right of whi
                        cfull = (S - whi) // 512
                        rrlen = (S - whi) - 512 * cfull
                        if cfull == 0 and rrlen == 0:
                            nc.vector.memset(sL[:, 1:2], 0.0)
                        elif rrlen == 0:
                            if cfull == 1:
                                nc.vector.tensor_copy(sL[:, 1:2], acc[:, 3:4])
                            else:
                                nc.vector.tensor_reduce(
                                    sL[:, 1:2], acc[:, 4 - cfull:4], axis=AX.X,
                                    op=ALU.add)
                        else:
                            nc.vector.tensor_reduce(
                                sL[:, 1:2], ex[:, whi:whi + rrlen], axis=AX.X,
                                op=ALU.add)
                            if cfull == 1:
                                nc.vector.tensor_add(
                                    sL[:, 1:2], sL[:, 1:2], acc[:, 3:4])
                            elif cfull > 1:
                                tmp = smal.tile([128, 1], F32, tag="tmp1",
                                                name="tmp1")
                                nc.vector.tensor_reduce(
                                    tmp[:], acc[:, 4 - cfull:4], axis=AX.X,
                                    op=ALU.add)
                                nc.vector.tensor_add(sL[:, 1:2], sL[:, 1:2],
                                                     tmp[:])

                        # arel edge cols: left into arel16, right into pack8
                        nc.gpsimd.tensor_add(arel16[:, h, 0:1], sL[:, 0:1],
                                             tails[:, 0:1])
                        nc.gpsimd.tensor_add(pack8[:, 4 + h:5 + h], sL[:, 1:2],
                                             tails[:, 1:2])

                        # D and reciprocal
                        Dt = smal.tile([128, 1], F32, tag=f"Dt{h}",
                                       name=f"Dt{h}")
                        nc.vector.tensor_reduce(
                            Dt[:], acc[:], axis=AX.X, op=ALU.add)
                        nc.vector.reciprocal(recips[:, h:h + 1], Dt[:])

                # ---- arel center: diagonal read for all 4 heads
                nc.sync.dma_start(
                    arel16[:, :, 1:128],
                    AP(imgw_t, wbase(t) + (i0 - wlo) + 1,
                       [[IMWW + 1, 128], [128 * IMWW, NH], [1, 127]]))

                # ---- PE transposes: attn blocks, arel, recips
                for h in range(NH):
                    ex = exps[h]
                    for j in range(4):
                        tp = tpps.tile([128, 512], F16, tag="tp", name="tp")
                        for kk in range(4):
                            kt = 4 * j + kk
                            nc.tensor.transpose(
                                tp[:, 128 * kk:128 * kk + 128],
                                ex[:, 128 * kt:128 * kt + 128], id_sb[:])
                        nc.vector.tensor_copy(
                            attnT[h][:, 4 * j:4 * j + 4,
                                     128 * tq:128 * tq + 128],
                            tp[:].rearrange("p (a b) -> p a b", a=4))
                    tpa = tpps.tile([128, 512], F16, tag="tp", name="tp")
                    nc.tensor.transpose(
                        tpa[:, 0:128], arel16[:, h, 0:128], id_sb[:])
                    nc.vector.tensor_copy(
                        arelT[h][:, 128 * tq:128 * tq + 128], tpa[:, 0:128])

                nc.gpsimd.tensor_copy(pack8[:, 0:4], recips[:])
                for h in range(NH):
                    o = 512 * h + 128 * tq
                    nc.sync.dma_start(recipT16[0:1, o:o + 128],
                                      pack8[:, h:h + 1])
                    nc.sync.dma_start(arelR[0:1, o:o + 128],
                                      pack8[:, 4 + h:5 + h])

            # ---------------- per-group AV + rel-v + normalize + out ----
            ctx16 = {}
            for pair in range(2):
                bc_ps = smps.tile([128, 512], F32, tag="sm", name="sm")
                for h01 in range(2):
                    h = 2 * pair + h01
                    nc.tensor.matmul(
                        bc_ps[64 * h01:64 * h01 + 64, :], ones1[0:1, 0:64],
                        recipT16[0:1, 512 * h:512 * h + 512], start=True,
                        stop=True, tile_position=(0, 64 * h01))
                rbc = big2.tile([128, 512], F16, tag="rbc", name="rbc")
                nc.vector.tensor_copy(rbc[:], bc_ps[:])
                ctx_ps = avps.tile([128, 512], F32, tag=f"av{pair}",
                                   name=f"av{pair}")
                for h01 in range(2):
                    h = 2 * pair + h01
                    cs = slice(64 * h01, 64 * h01 + 64)
                    tp_ = (0, 64 * h01)
                    for kt in range(NKT):
                        nc.tensor.matmul(
                            ctx_ps[cs, :], v16[:, kt, 64 * h:64 * h + 64],
                            attnT[h][:, kt, :], start=(kt == 0), stop=False,
                            tile_position=tp_)
                    nc.tensor.matmul(
                        ctx_ps[cs, :], rvm_sb[:, :], arelT[h][:, :],
                        start=False, stop=False, tile_position=tp_)
                    nc.tensor.matmul(
                        ctx_ps[cs, :], rvl_sb[0:1, :],
                        arelR[0:1, 512 * h:512 * h + 512],
                        start=False, stop=True, tile_position=tp_)

                ct = p2.tile([128, 512], F16, tag=f"ctx16_{pair}",
                             name=f"ctx16_{pair}")
                nc.vector.tensor_tensor(ct[:], ctx_ps[:], rbc[:],
                                        op=ALU.mult)
                nc.vector.tensor_scalar_add(ct[:], ct[:],
                                            bv_sb[:, pair:pair + 1])
                ctx16[pair] = ct

            # ---- output projection for this q512 group
            for tq in range(4):
                out_sb = big2.tile([128, 1024], F32, tag="out_sb",
                                   name="out_sb")
                for nch in range(2):
                    op_ps = smps.tile([128, 512], F32, tag="sm", name="sm")
                    for pair in range(2):
                        nc.tensor.matmul(
                            op_ps[:], ctx16[pair][:, 128 * tq:128 * tq + 128],
                            wout_sb[:, pair, 512 * nch:512 * nch + 512],
                            start=(pair == 0), stop=(pair == 1))
                    nc.vector.tensor_copy(
                        out_sb[:, 512 * nch:512 * nch + 512], op_ps[:])
                r0_ = 512 * go + 128 * tq
                nc.sync.dma_start(out[r0_:r0_ + 128, :], out_sb[:])

    nc.compile()
    return nc


def get_nc():
    if "nc" not in _cache:
        _cache["nc"] = _build()
    return _cache["nc"]


def shard_inputs(inputs):
    """Build per-core input maps from full inputs (layout prep only)."""
    x = np.asarray(inputs["x"], np.float32)
    W_qkv = np.asarray(inputs["W_qkv"], np.float32)
    b_qkv = np.asarray(inputs["b_qkv"], np.float32)
    W_out = np.asarray(inputs["W_out"], np.float32)
    rk = np.asarray(inputs["rel_emb_k"], np.float32)
    rv = np.asarray(inputs["rel_emb_v"], np.float32)

    Wq, Wk, Wv = W_qkv[:, 0:D], W_qkv[:, D:2 * D], W_qkv[:, 2 * D:3 * D]
    bqf, bkf, bvf = b_qkv[0:D], b_qkv[D:2 * D], b_qkv[2 * D:3 * D]

    relk_host = np.zeros((128, 132), np.float16)
    relk_host[0:64, 0:129] = rk.T.astype(np.float16)
    relk_host[64:128, 0:129] = rk.T.astype(np.float16)
    rvm_host = rv[0:128].astype(np.float16)
    rvl_host = rv[128:129].astype(np.float16)
    r0 = np.tile(rk[0], 2).reshape(128, 1)
    r1 = np.tile(rk[128], 2).reshape(128, 1)
    r01_host = np.concatenate([r0, r1], 1).astype(np.float32)

    # tail masks [128, 512] master: middle tiles slice [0:wlen],
    # t=0 slices [128:128+wlen] (equivalent to off=0 masks)
    jj = np.arange(512)[None, :]
    ppi = np.arange(128)[:, None]
    ml_host = (jj <= ppi + 128 - 64).astype(np.float16)
    mr_host = (jj >= ppi + 128 + 64).astype(np.float16)
    ident_host = np.eye(128, dtype=np.float16)

    in_maps = []
    for c in range(N_CORES):
        b, g = c // 4, c % 4
        cols = slice(256 * g, 256 * g + 256)
        m = {
            "xT": np.ascontiguousarray(x[b].T),
            "wq": np.ascontiguousarray(Wq[:, cols]),
            "wk": np.ascontiguousarray(Wk[:, cols]),
            "wv": np.ascontiguousarray(Wv[:, cols]),
            "bq": np.ascontiguousarray(bqf[cols].reshape(2, 128).T),
            "bk": np.ascontiguousarray(bkf[cols].reshape(2, 128).T),
            "bv": np.ascontiguousarray(bvf[cols].reshape(2, 128).T),
            "r01": r01_host,
            "relk": relk_host,
            "rvm": rvm_host,
            "rvl": rvl_host,
            "wout": np.ascontiguousarray(
                W_out[cols].reshape(2, 128, 1024).transpose(1, 0, 2)
            ).astype(np.float16),
            "mlm": ml_host,
            "mrm": mr_host,
            "ident": ident_host,
        }
        in_maps.append(m)
    return in_maps


def unshard_outputs(results, inputs):
    b_out = np.asarray(inputs["b_out"], np.float32)
    out = np.zeros((B, S, D), np.float32)
    for c in range(N_CORES):
        out[c // 4] += results[c]["out"].astype(np.float32)
    out += b_out[None, None, :]
    return out


def kernel(**inputs):
    from concourse import bass_utils
    nc = get_nc()
    in_maps = shard_inputs(inputs)
    res = bass_utils.run_bass_kernel_spmd(nc, in_maps, list(range(N_CORES)))
    return unshard_outputs(res.results, inputs)


if __name__ == "__main__":
    rng = np.random.default_rng(0)
    demo = {
        "x": rng.standard_normal((B, S, D)).astype(np.float32),
        "W_qkv": (rng.standard_normal((D, 3 * D)) * 0.02).astype(np.float32),
        "b_qkv": np.zeros(3 * D, np.float32),
        "W_out": (rng.standard_normal((D, D)) * 0.02).astype(np.float32),
        "b_out": np.zeros(D, np.float32),
        "rel_emb_k": (rng.standard_normal((VOC, HD)) * 0.02).astype(np.float32),
        "rel_emb_v": (rng.standard_normal((VOC, HD)) * 0.02).astype(np.float32),
    }
    o = kernel(**demo)
    print(o.shape, float(np.abs(o).max()))


# revision 16
# speedup vs baseline: 1.7206x; 1.4449x over previous
"""Trainium2 Bass kernel for MultiHeadedSelfAttention with Shaw relative
position embeddings (clipped, R=64), sharded over 8 NeuronCores.

Sharding: core c handles batch b = c//4 and head group g = c%4 (4 heads).
Each core computes a partial output  ctx_g @ W_out[256g:256g+256]  for its
batch; the host sums the 4 partials per batch and adds b_out.

v2: PE-array transposes of the attention matrix (replacing DMA
transposes), mask-based softmax tail sums (replacing the cumsum image),
merged/multi-engine DMA dispatch.
"""
import sys

sys.path.insert(0, "/opt/trn_rl_repo")

import numpy as np

B, S, D, H, RR, VOC = 2, 2048, 1024, 16, 64, 129
HD = 64              # head dim
NH = 4               # heads per core
N_CORES = 8
NT = S // 128        # 16 q-tiles of 128
NKT = S // 128       # 16 k-tiles
IMW = 512            # qrel image width (clip-padded)
IMWW = 448           # exp W-zone image width (64-col front pad)
FPAD = 64            # front pad of the exp image
SCALE = 0.125        # 1/sqrt(64)

_cache = {}


def _regions(t):
    """W-zone bounds for q-tile t."""
    i0 = 128 * t
    wlo = max(0, i0 - 128)
    whi = 256 if t == 0 else min(S, i0 + 256)
    return i0, wlo, whi


def _build():
    import concourse.bass as bass
    import concourse.mybir as mybir
    import concourse.tile as tile
    from concourse import bacc
    from contextlib import ExitStack

    F32 = mybir.dt.float32
    F32R = mybir.dt.float32r
    F16 = mybir.dt.float16
    AP = bass.AP
    AF = mybir.ActivationFunctionType
    ALU = mybir.AluOpType
    AX = mybir.AxisListType

    nc = bacc.Bacc("TRN2", target_bir_lowering=False, debug=False,
                   num_devices=N_CORES)

    # ---------------- DRAM I/O ----------------
    xT = nc.dram_tensor("xT", [D, S], F32, kind="ExternalInput").ap()
    wq = nc.dram_tensor("wq", [D, 256], F32, kind="ExternalInput").ap()
    wk = nc.dram_tensor("wk", [D, 256], F32, kind="ExternalInput").ap()
    wv = nc.dram_tensor("wv", [D, 256], F32, kind="ExternalInput").ap()
    bq = nc.dram_tensor("bq", [128, 2], F32, kind="ExternalInput").ap()
    bk = nc.dram_tensor("bk", [128, 2], F32, kind="ExternalInput").ap()
    bv = nc.dram_tensor("bv", [128, 2], F32, kind="ExternalInput").ap()
    r01 = nc.dram_tensor("r01", [128, 2], F32, kind="ExternalInput").ap()
    relk = nc.dram_tensor("relk", [128, 132], F16, kind="ExternalInput").ap()
    rvm = nc.dram_tensor("rvm", [128, 64], F16, kind="ExternalInput").ap()
    rvl = nc.dram_tensor("rvl", [1, 64], F16, kind="ExternalInput").ap()
    wout = nc.dram_tensor("wout", [128, 2, 1024], F16, kind="ExternalInput").ap()
    # tail masks: [ML | MR] x [t==0 | middle] fp16 [128, 384]
    mlm = nc.dram_tensor("mlm", [128, 512], F16, kind="ExternalInput").ap()
    mrm = nc.dram_tensor("mrm", [128, 512], F16, kind="ExternalInput").ap()
    ident = nc.dram_tensor("ident", [128, 128], F16, kind="ExternalInput").ap()
    out = nc.dram_tensor("out", [S, D], F32, kind="ExternalOutput").ap()

    # DRAM scratch images
    imgq_t = nc.dram_tensor("imgq", [NT * NH * 128 * IMW], F16)   # qrel pad
    imgw_t = nc.dram_tensor("imgw", [NT * NH * 128 * IMWW], F16)  # exp W-zone

    def qbase(t):
        return t * NH * 128 * IMW

    def wbase(t, h=0):
        return (t * NH + h) * 128 * IMWW

    with tile.TileContext(nc) as tc, ExitStack() as ctx:
        # ---------------- persistent pools ----------------
        pp = ctx.enter_context(tc.tile_pool(name="persist", bufs=1))
        qkT = []   # per pair: qT16, kW16, kL16, kR16  [128, S] fp16
        for pair in range(2):
            qkT.append({
                "q": pp.tile([128, S], F16, tag=f"qT{pair}", name=f"qT{pair}"),
                "W": pp.tile([128, S], F16, tag=f"kW{pair}", name=f"kW{pair}"),
                "L": pp.tile([128, S], F16, tag=f"kL{pair}", name=f"kL{pair}"),
                "R": pp.tile([128, S], F16, tag=f"kR{pair}", name=f"kR{pair}"),
            })
        v16 = pp.tile([128, NKT, 256], F16, tag="v16", name="v16")
        relk_sb = pp.tile([128, 132], F16, tag="relk", name="relk")
        rvm_sb = pp.tile([128, 64], F16, tag="rvm", name="rvm")
        rvl_sb = pp.tile([1, 64], F16, tag="rvl", name="rvl")
        wout_sb = pp.tile([128, 2, 1024], F16, tag="wout", name="wout")
        bq_sb = pp.tile([128, 2], F32, tag="bq", name="bq")
        bk_sb = pp.tile([128, 2], F32, tag="bk", name="bk")
        bv_sb = pp.tile([128, 2], F32, tag="bv", name="bv")
        r01_sb = pp.tile([128, 2], F32, tag="r01", name="r01")
        ml_sb = pp.tile([128, 512], F16, tag="ml", name="ml")
        mr_sb = pp.tile([128, 512], F16, tag="mr", name="mr")
        id_sb = pp.tile([128, 128], F16, tag="ident", name="ident")
        ones1 = pp.tile([1, 128], F16, tag="ones1", name="ones1")
        zeros16 = pp.tile([128, 128], F16, tag="zeros16", name="zeros16")
        attnT = [pp.tile([128, NKT, 512], F16, tag=f"attnT{h}", name=f"attnT{h}")
                 for h in range(NH)]
        arelT = [pp.tile([128, 512], F16, tag=f"arelT{h}", name=f"arelT{h}")
                 for h in range(NH)]

        nc.sync.dma_start(relk_sb[:], relk)
        nc.sync.dma_start(rvm_sb[:], rvm)
        nc.sync.dma_start(rvl_sb[:], rvl)
        nc.sync.dma_start(wout_sb[:], wout)
        nc.sync.dma_start(bq_sb[:], bq)
        nc.sync.dma_start(bk_sb[:], bk)
        nc.sync.dma_start(bv_sb[:], bv)
        nc.sync.dma_start(r01_sb[:], r01)
        nc.sync.dma_start(ml_sb[:], mlm)
        nc.sync.dma_start(mr_sb[:], mrm)
        nc.sync.dma_start(id_sb[:], ident)
        nc.gpsimd.memset(ones1[:], 1.0)
        nc.gpsimd.memset(zeros16[:], 0.0)
        # zero the t=0 front pad (cols [0,64)) and t=15 back pad
        # (cols [320,448)) of the exp images so diagonal reads see 0
        for h in range(NH):
            nc.sync.dma_start(
                AP(imgw_t, wbase(0, h), [[IMWW, 128], [1, FPAD]]),
                zeros16[:, 0:FPAD])
            nc.sync.dma_start(
                AP(imgw_t, wbase(15, h) + FPAD + 256,
                   [[IMWW, 128], [1, 128]]),
                zeros16[:, 0:128])

        # ---------------- phase 1: projections ----------------
        with tc.tile_pool(name="p1", bufs=1) as p1, \
             tc.tile_pool(name="p1ps", bufs=2, space="PSUM") as p1ps:
            xT_sb = p1.tile([128, 8, S], F32R, tag="xT", name="xT")
            wq_sb = p1.tile([128, 8, 256], F32R, tag="wq", name="wq")
            wk_sb = p1.tile([128, 8, 256], F32R, tag="wk", name="wk")
            wv_sb = p1.tile([128, 8, 256], F32R, tag="wv", name="wv")
            nc.sync.dma_start(xT_sb[:], xT.rearrange("(c p) s -> p c s", p=128).bitcast(F32R))
            nc.sync.dma_start(wq_sb[:], wq.rearrange("(c p) n -> p c n", p=128).bitcast(F32R))
            nc.sync.dma_start(wk_sb[:], wk.rearrange("(c p) n -> p c n", p=128).bitcast(F32R))
            nc.sync.dma_start(wv_sb[:], wv.rearrange("(c p) n -> p c n", p=128).bitcast(F32R))

            # q, k (transposed layout [col, s]) per pair
            for pair in range(2):
                for sc in range(4):  # s-chunks of 512
                    ps_q = p1ps.tile([128, 512], F32, tag="p1q", name="p1q")
                    ps_k = p1ps.tile([128, 512], F32, tag="p1k", name="p1k")
                    for dk in range(8):
                        nc.tensor.matmul(
                            ps_q[:], wq_sb[:, dk, 128 * pair:128 * pair + 128],
                            xT_sb[:, dk, 512 * sc:512 * sc + 512],
                            start=(dk == 0), stop=(dk == 7))
                        nc.tensor.matmul(
                            ps_k[:], wk_sb[:, dk, 128 * pair:128 * pair + 128],
                            xT_sb[:, dk, 512 * sc:512 * sc + 512],
                            start=(dk == 0), stop=(dk == 7))
                    cs = slice(512 * sc, 512 * sc + 512)
                    nc.vector.tensor_scalar(
                        qkT[pair]["q"][:, cs], ps_q[:], bq_sb[:, pair:pair + 1],
                        SCALE, op0=ALU.add, op1=ALU.mult)
                    nc.vector.tensor_scalar_add(
                        qkT[pair]["W"][:, cs], ps_k[:], bk_sb[:, pair:pair + 1])
                    nc.vector.tensor_scalar_add(
                        qkT[pair]["L"][:, cs], qkT[pair]["W"][:, cs],
                        r01_sb[:, 0:1])
                    nc.vector.tensor_scalar_add(
                        qkT[pair]["R"][:, cs], qkT[pair]["W"][:, cs],
                        r01_sb[:, 1:2])

            # v (natural layout [s, col])
            for st in range(NT):
                ps_v = p1ps.tile([128, 256], F32, tag="p1v", name="p1v")
                for dk in range(8):
                    nc.tensor.matmul(
                        ps_v[:], xT_sb[:, dk, 128 * st:128 * st + 128],
                        wv_sb[:, dk, :], start=(dk == 0), stop=(dk == 7))
                nc.vector.tensor_copy(v16[:, st, :], ps_v[:])

        # ---------------- phase 1.5: qrel images for all tiles ----------
        qrp = ctx.enter_context(tc.tile_pool(name="qrp", bufs=2))
        qrps = ctx.enter_context(tc.tile_pool(name="qrps", bufs=1, space="PSUM"))
        for t in range(NT):
            qrelpad = qrp.tile([128, NH, IMW], F16, tag="qrelpad", name="qrelpad")
            for pair in range(2):
                for h01 in range(2):
                    h = 2 * pair + h01
                    rs = slice(64 * h01, 64 * h01 + 64)
                    qr = qrps.tile([128, 132], F32, tag="qr", name="qr")
                    nc.tensor.matmul(
                        qr[:], qkT[pair]["q"][rs, 128 * t:128 * t + 128],
                        relk_sb[rs, :], start=True, stop=True)
                    nc.vector.tensor_copy(qrelpad[:, h, 192:321], qr[:, 0:129])
                    nc.vector.tensor_copy(
                        qrelpad[:, h, 0:192],
                        qr[:, 0:1].broadcast_to([128, 192]))
                    nc.vector.tensor_copy(
                        qrelpad[:, h, 321:IMW],
                        qr[:, 128:129].broadcast_to([128, IMW - 321]))
            nc.sync.dma_start(
                AP(imgq_t, qbase(t), [[IMW, 128], [128 * IMW, NH], [1, IMW]]),
                qrelpad[:])

        # ---------------- phase 2: attention ----------------
        p2 = ctx.enter_context(tc.tile_pool(name="p2", bufs=1))
        scps = ctx.enter_context(tc.tile_pool(name="scps", bufs=2, space="PSUM"))
        tpps = ctx.enter_context(tc.tile_pool(name="tpps", bufs=2, space="PSUM"))
        avps = ctx.enter_context(tc.tile_pool(name="avps", bufs=1, space="PSUM"))
        smps = ctx.enter_context(tc.tile_pool(name="smps", bufs=1, space="PSUM"))
        expp = ctx.enter_context(tc.tile_pool(name="expp", bufs=2))
        smal = ctx.enter_context(tc.tile_pool(name="smal", bufs=4))
        big2 = ctx.enter_context(tc.tile_pool(name="big2", bufs=2))
        bndp = ctx.enter_context(tc.tile_pool(name="bndp", bufs=2))

        for go in range(4):            # q512 groups
            recipT16 = big2.tile([1, NH * 512], F16, tag="recipT16",
                                 name="recipT16")
            arelR = big2.tile([1, NH * 512], F16, tag="arelR", name="arelR")
            for tq in range(4):        # q128 tiles within group
                t = 4 * go + tq
                i0, wlo, whi = _regions(t)
                wlen = whi - wlo
                moff = 128 if t == 0 else 0   # t=0 masks = middle masks shifted

                # ---- band read (from qrel image): all 4 heads in one DMA
                band4 = bndp.tile([128, NH, 384], F16, tag="band4", name="band4")
                nc.sync.dma_start(
                    band4[:, :, 0:wlen],
                    AP(imgq_t, qbase(t) + 256 - (i0 - wlo),
                       [[IMW - 1, 128], [128 * IMW, NH], [1, wlen]]))

                recips = smal.tile([128, 4], F32, tag="recips", name="recips")
                pack8 = smal.tile([128, 8], F16, tag="pack8", name="pack8")
                arel16 = smal.tile([128, NH, 129], F16, tag="arel16",
                                   name="arel16")
                exps = {}
                for pair in range(2):
                    for h01 in range(2):
                        h = 2 * pair + h01
                        rs = slice(64 * h01, 64 * h01 + 64)
                        ex = expp.tile([128, S], F16, tag=f"exp{h}",
                                       name=f"exp{h}")
                        exps[h] = ex
                        acc = smal.tile([128, 4], F32, tag=f"acc{h}",
                                        name=f"acc{h}")

                        # ---- scores + band + exp, per 512-col chunk
                        for c in range(4):
                            clo, chi = 512 * c, 512 * c + 512
                            sc_ps = scps.tile([128, 512], F32, tag="sc",
                                              name="sc")
                            pieces = []
                            for rlo, rhi, key in ((0, wlo, "L"), (wlo, whi, "W"),
                                                  (whi, S, "R")):
                                lo, hi = max(rlo, clo), min(rhi, chi)
                                if lo < hi:
                                    pieces.append((lo, hi, key))
                            for lo, hi, key in pieces:
                                nc.tensor.matmul(
                                    sc_ps[:, lo - clo:hi - clo],
                                    qkT[pair]["q"][rs, 128 * t:128 * t + 128],
                                    qkT[pair][key][rs, lo:hi],
                                    start=True, stop=True)
                            blo, bhi = max(wlo, clo), min(whi, chi)
                            if blo < bhi:
                                nc.vector.tensor_add(
                                    sc_ps[:, blo - clo:bhi - clo],
                                    sc_ps[:, blo - clo:bhi - clo],
                                    band4[:, h, blo - wlo:bhi - wlo])
                            nc.scalar.activation(
                                ex[:, clo:chi], sc_ps[:], AF.Exp,
                                accum_out=acc[:, c:c + 1])

                        # ---- write exp W-zone to image (for arel diag read)
                        nc.sync.dma_start(
                            AP(imgw_t, wbase(t, h) + FPAD,
                               [[IMWW, 128], [1, wlen]]),
                            ex[:, wlo:whi])

                        # ---- masked tail sums
                        tails = smal.tile([128, 2], F32, tag=f"tails{h}",
                                          name=f"tails{h}")
                        scr = smal.tile([128, 384], F16, tag="scr", name="scr")
                        nc.vector.tensor_tensor(
                            scr[:, 0:wlen], ex[:, wlo:whi],
                            ml_sb[:, moff:moff + wlen], op=ALU.mult)
                        nc.vector.tensor_reduce(
                            tails[:, 0:1], scr[:, 0:wlen], axis=AX.X,
                            op=ALU.add)
                        scr2 = smal.tile([128, 384], F16, tag="scr", name="scr")
                        nc.vector.tensor_tensor(
                            scr2[:, 0:wlen], ex[:, wlo:whi],
                            mr_sb[:, moff:moff + wlen], op=ALU.mult)
                        nc.vector.tensor_reduce(
                            tails[:, 1:2], scr2[:, 0:wlen], axis=AX.X,
                            op=ALU.add)

                        # sL = full chunks + partial chunk left of wlo
                        sL = smal.tile([128, 2], F32, tag=f"sLR{h}",
                                       name=f"sLR{h}")
                        # sL: gpsimd accumulates full-chunk accs; DVE only
                        # for the partial-chunk reduce
                        nfull = wlo // 512
                        rlen = wlo - 512 * nfull
                        if rlen > 0:
                            nc.vector.tensor_reduce(
                                sL[:, 0:1], ex[:, 512 * nfull:wlo], axis=AX.X,
                                op=ALU.add)
                        else:
                            nc.gpsimd.memset(sL[:, 0:1], 0.0)
                        for cidx in range(nfull):
                            nc.gpsimd.tensor_add(
                                sL[:, 0:1], sL[:, 0:1], acc[:, cidx:cidx + 1])
                        cfull = (S - whi) // 512
                        rrlen = (S - whi) - 512 * cfull
                        if rrlen > 0:
                            nc.vector.tensor_reduce(
                                sL[:, 1:2], ex[:, whi:whi + rrlen], axis=AX.X,
                                op=ALU.add)
                        else:
                            nc.gpsimd.memset(sL[:, 1:2], 0.0)
                        for cidx in range(4 - cfull, 4):
                            nc.gpsimd.tensor_add(
                                sL[:, 1:2], sL[:, 1:2], acc[:, cidx:cidx + 1])

                        # arel edge cols: left into arel16, right into pack8
                        nc.gpsimd.tensor_add(arel16[:, h, 0:1], sL[:, 0:1],
                                             tails[:, 0:1])
                        nc.gpsimd.tensor_add(pack8[:, 4 + h:5 + h], sL[:, 1:2],
                                             tails[:, 1:2])

                        # D and reciprocal
                        Dt = smal.tile([128, 1], F32, tag=f"Dt{h}",
                                       name=f"Dt{h}")
                        nc.gpsimd.tensor_add(Dt[:], acc[:, 0:1], acc[:, 1:2])
                        nc.gpsimd.tensor_add(Dt[:], Dt[:], acc[:, 2:3])
                        nc.gpsimd.tensor_add(Dt[:], Dt[:], acc[:, 3:4])
                        nc.vector.reciprocal(recips[:, h:h + 1], Dt[:])

                # ---- arel center: diagonal read for all 4 heads
                nc.sync.dma_start(
                    arel16[:, :, 1:128],
                    AP(imgw_t, wbase(t) + (i0 - wlo) + 1,
                       [[IMWW + 1, 128], [128 * IMWW, NH], [1, 127]]))

                # ---- PE transposes: attn blocks, arel, recips
                for h in range(NH):
                    ex = exps[h]
                    for j in range(2):
                        tp = tpps.tile([128, 1024], F16, tag="tp", name="tp")
                        for kk in range(8):
                            kt = 8 * j + kk
                            nc.tensor.transpose(
                                tp[:, 128 * kk:128 * kk + 128],
                                ex[:, 128 * kt:128 * kt + 128], id_sb[:])
                        nc.vector.tensor_copy(
                            attnT[h][:, 8 * j:8 * j + 8,
                                     128 * tq:128 * tq + 128],
                            tp[:].rearrange("p (a b) -> p a b", a=8))
                    tpa = tpps.tile([128, 1024], F16, tag="tp", name="tp")
                    nc.tensor.transpose(
                        tpa[:, 0:128], arel16[:, h, 0:128], id_sb[:])
                    nc.vector.tensor_copy(
                        arelT[h][:, 128 * tq:128 * tq + 128], tpa[:, 0:128])

                nc.gpsimd.tensor_copy(pack8[:, 0:4], recips[:])
                for h in range(NH):
                    o = 512 * h + 128 * tq
                    nc.sync.dma_start(recipT16[0:1, o:o + 128],
                                      pack8[:, h:h + 1])
                    nc.sync.dma_start(arelR[0:1, o:o + 128],
                                      pack8[:, 4 + h:5 + h])

            # ---------------- per-group AV + rel-v + normalize + out ----
            ctx16 = {}
            for pair in range(2):
                bc_ps = smps.tile([128, 512], F32, tag="sm", name="sm")
                for h01 in range(2):
                    h = 2 * pair + h01
                    nc.tensor.matmul(
                        bc_ps[64 * h01:64 * h01 + 64, :], ones1[0:1, 0:64],
                        recipT16[0:1, 512 * h:512 * h + 512], start=True,
                        stop=True, tile_position=(0, 64 * h01))
                rbc = big2.tile([128, 512], F16, tag="rbc", name="rbc")
                nc.vector.tensor_copy(rbc[:], bc_ps[:])
                ctx_ps = avps.tile([128, 512], F32, tag=f"av{pair}",
                                   name=f"av{pair}")
                for h01 in range(2):
                    h = 2 * pair + h01
                    cs = slice(64 * h01, 64 * h01 + 64)
                    tp_ = (0, 64 * h01)
                    for kt in range(NKT):
                        nc.tensor.matmul(
                            ctx_ps[cs, :], v16[:, kt, 64 * h:64 * h + 64],
                            attnT[h][:, kt, :], start=(kt == 0), stop=False,
                            tile_position=tp_)
                    nc.tensor.matmul(
                        ctx_ps[cs, :], rvm_sb[:, :], arelT[h][:, :],
                        start=False, stop=False, tile_position=tp_)
                    nc.tensor.matmul(
                        ctx_ps[cs, :], rvl_sb[0:1, :],
                        arelR[0:1, 512 * h:512 * h + 512],
                        start=False, stop=True, tile_position=tp_)

                ct = p2.tile([128, 512], F16, tag=f"ctx16_{pair}",
                             name=f"ctx16_{pair}")
                nc.vector.tensor_tensor(ct[:], ctx_ps[:], rbc[:],
                                        op=ALU.mult)
                nc.vector.tensor_scalar_add(ct[:], ct[:],
                                            bv_sb[:, pair:pair + 1])
                ctx16[pair] = ct

            # ---- output projection for this q512 group
            for tq in range(4):
                out_sb = big2.tile([128, 1024], F32, tag="out_sb",
                                   name="out_sb")
                for nch in range(2):
                    op_ps = smps.tile([128, 512], F32, tag="sm", name="sm")
                    for pair in range(2):
                        nc.tensor.matmul(
                            op_ps[:], ctx16[pair][:, 128 * tq:128 * tq + 128],
                            wout_sb[:, pair, 512 * nch:512 * nch + 512],
                            start=(pair == 0), stop=(pair == 1))
                    nc.vector.tensor_copy(
                        out_sb[:, 512 * nch:512 * nch + 512], op_ps[:])
                r0_ = 512 * go + 128 * tq
                nc.sync.dma_start(out[r0_:r0_ + 128, :], out_sb[:])

    nc.compile()
    return nc


def get_nc():
    if "nc" not in _cache:
        _cache["nc"] = _build()
    return _cache["nc"]


def shard_inputs(inputs):
    """Build per-core input maps from full inputs (layout prep only)."""
    x = np.asarray(inputs["x"], np.float32)
    W_qkv = np.asarray(inputs["W_qkv"], np.float32)
    b_qkv = np.asarray(inputs["b_qkv"], np.float32)
    W_out = np.asarray(inputs["W_out"], np.float32)
    rk = np.asarray(inputs["rel_emb_k"], np.float32)
    rv = np.asarray(inputs["rel_emb_v"], np.float32)

    Wq, Wk, Wv = W_qkv[:, 0:D], W_qkv[:, D:2 * D], W_qkv[:, 2 * D:3 * D]
    bqf, bkf, bvf = b_qkv[0:D], b_qkv[D:2 * D], b_qkv[2 * D:3 * D]

    relk_host = np.zeros((128, 132), np.float16)
    relk_host[0:64, 0:129] = rk.T.astype(np.float16)
    relk_host[64:128, 0:129] = rk.T.astype(np.float16)
    rvm_host = rv[0:128].astype(np.float16)
    rvl_host = rv[128:129].astype(np.float16)
    r0 = np.tile(rk[0], 2).reshape(128, 1)
    r1 = np.tile(rk[128], 2).reshape(128, 1)
    r01_host = np.concatenate([r0, r1], 1).astype(np.float32)

    # tail masks [128, 512] master: middle tiles slice [0:wlen],
    # t=0 slices [128:128+wlen] (equivalent to off=0 masks)
    jj = np.arange(512)[None, :]
    ppi = np.arange(128)[:, None]
    ml_host = (jj <= ppi + 128 - 64).astype(np.float16)
    mr_host = (jj >= ppi + 128 + 64).astype(np.float16)
    ident_host = np.eye(128, dtype=np.float16)

    in_maps = []
    for c in range(N_CORES):
        b, g = c // 4, c % 4
        cols = slice(256 * g, 256 * g + 256)
        m = {
            "xT": np.ascontiguousarray(x[b].T),
            "wq": np.ascontiguousarray(Wq[:, cols]),
            "wk": np.ascontiguousarray(Wk[:, cols]),
            "wv": np.ascontiguousarray(Wv[:, cols]),
            "bq": np.ascontiguousarray(bqf[cols].reshape(2, 128).T),
            "bk": np.ascontiguousarray(bkf[cols].reshape(2, 128).T),
            "bv": np.ascontiguousarray(bvf[cols].reshape(2, 128).T),
            "r01": r01_host,
            "relk": relk_host,
            "rvm": rvm_host,
            "rvl": rvl_host,
            "wout": np.ascontiguousarray(
                W_out[cols].reshape(2, 128, 1024).transpose(1, 0, 2)
            ).astype(np.float16),
            "mlm": ml_host,
            "mrm": mr_host,
            "ident": ident_host,
        }
        in_maps.append(m)
    return in_maps


def unshard_outputs(results, inputs):
    b_out = np.asarray(inputs["b_out"], np.float32)
    out = np.zeros((B, S, D), np.float32)
    for c in range(N_CORES):
        out[c // 4] += results[c]["out"].astype(np.float32)
    out += b_out[None, None, :]
    return out


def kernel(**inputs):
    from concourse import bass_utils
    nc = get_nc()
    in_maps = shard_inputs(inputs)
    res = bass_utils.run_bass_kernel_spmd(nc, in_maps, list(range(N_CORES)))
    return unshard_outputs(res.results, inputs)


if __name__ == "__main__":
    rng = np.random.default_rng(0)
    demo = {
        "x": rng.standard_normal((B, S, D)).astype(np.float32),
        "W_qkv": (rng.standard_normal((D, 3 * D)) * 0.02).astype(np.float32),
        "b_qkv": np.zeros(3 * D, np.float32),
        "W_out": (rng.standard_normal((D, D)) * 0.02).astype(np.float32),
        "b_out": np.zeros(D, np.float32),
        "rel_emb_k": (rng.standard_normal((VOC, HD)) * 0.02).astype(np.float32),
        "rel_emb_v": (rng.standard_normal((VOC, HD)) * 0.02).astype(np.float32),
    }
    o = kernel(**demo)
    print(o.shape, float(np.abs(o).max()))
